# revision 12
# baseline (speedup 1.0000x reference)
"""Sparse-attention (PVT-style SRA) kernel for 8 Trainium2 NeuronCores.

Sharding: 8 cores = 2 batches x 4 row-quarters of N=8000. Each core computes
its 2000 output rows end-to-end; the spatial-reduction branch (conv+LN+kv) is
replicated per batch. All matmuls run in fp16 (1 cycle/row on the PE, fast
weight load); accumulation stays fp32 in PSUM. The depthwise conv runs on the
tensor engine as diagonal-weight matmuls accumulating all 27 taps in PSUM;
the trilinear upsample is an interpolation matmul whose weights (products of
1/4 and 3/4) are fp16-exact. Host pre-transposes x into channel-major layout
so the kernel does no PE transposes of the input.

Phases: (1) conv + LN + kv + v transposes, (2) q projection + upsampled
identity branch + LN, (3) attention with an Exp-only scalar engine, then
normalize + output projection per row chunk.
"""

import sys

sys.path.insert(0, "/opt/trn_rl_repo")

import contextlib
import numpy as np
import concourse.bacc as bacc
import concourse.mybir as mybir
from concourse.tile import TileContext
from concourse.bass_utils import run_bass_kernel_spmd

dt = mybir.dt
Alu = mybir.AluOpType
Act = mybir.ActivationFunctionType

P = 128
B, N, C = 2, 8000, 512
H, HD = 8, 64
D3 = 20          # full spatial edge (D=H=W)
DR = 10          # reduced spatial edge
NSR = 1000       # DR**3
CT = 4           # C // P
NCHUNK = 2000    # output rows per core
RC = 4           # row chunks per core
RCW = 500        # rows per chunk
MT = 8           # key tiles
MTW = 125        # keys per tile
BW = 512         # PSUM bank width (f32 elems)
NTAP = 30        # 27 taps + 3 negated wrap-compensation taps
SCALE = HD ** -0.5
EPS = 1e-6

# dh==-1 and dw==-1 taps get wrap compensation with negated weights at
# diag slots 27..29 (tap index t -> neg slot).
_NEG = {0: 27, 9: 28, 18: 29}

_PROGRAM = None
_HOST = None
TRACE = False
LAST_RESULT = None


def _conv_ops(xf, acc_h, h):
    """Matmul op list for one 500-col half of the conv output block.

    The host permutes x's spatial rows into parity-block order: flat index
    (a*4+b*2+c)*1000 + d*100 + h*10 + w  <->  original (2d+a, 2h+b, 2w+c).
    Every tap then reduces to a shifted access pattern on xf, and the conv
    becomes a sum over taps of diag(w_tap) @ x_shifted, accumulated in PSUM.
    The center tap comes first and covers the whole half, so it owns
    start=True. Returns [(out_ap, in_ap, diag_slot)].

    xf: (p, 8000) fp16 input view; acc_h: (p, 500) f32 PSUM AP for output
    columns [h*500, (h+1)*500) of the 1000-col reduced block.
    """
    c0 = h * RCW
    c1 = c0 + RCW
    ops = []
    taps = [(0, 0, 0)] + [(a, b, c)
                          for a in (-1, 0, 1) for b in (-1, 0, 1) for c in (-1, 0, 1)
                          if (a, b, c) != (0, 0, 0)]
    for (dd, dh, dw) in taps:
        pa, pb, pc = (0 if dd == 0 else 1), (0 if dh == 0 else 1), (0 if dw == 0 else 1)
        Dd, Dh = (-1 if dd == -1 else 0), (-1 if dh == -1 else 0)
        bb = (pa * 4 + pb * 2 + pc) * 1000
        d0 = 1 if dd == -1 else 0
        t = (dd + 1) * 9 + (dh + 1) * 3 + (dw + 1)
        if dh != -1 and dw != -1:
            # contiguous: out o gets in bb + Dd*100 + o, o in [d0*100, 1000)
            a0 = max(c0, d0 * 100)
            ib = bb + Dd * 100 + a0
            ops.append((acc_h[:, a0 - c0:c1 - c0], xf[:, ib:ib + (c1 - a0)], t))
        elif dw == -1 and dh != -1:
            # out (x, w in 1..10), x = d*10+h_ in [d0*10, 100);
            # in = bb + Dd*100 + x*10 + (w-1)
            x0 = max(c0 // 10, d0 * 10)
            x1 = c1 // 10
            o = acc_h.rearrange("p (x w) -> p x w", w=10)[
                :, x0 - c0 // 10:x1 - c0 // 10, 1:10]
            ib = bb + Dd * 100 + x0 * 10
            i = xf[:, ib:ib + (x1 - x0) * 10].rearrange(
                "p (x w) -> p x w", w=10)[:, :, 0:9]
            ops.append((o, i, t))
        elif dh == -1 and dw != -1:
            # out (d, r in 10..100), d in [d0, 10); in = bb + Dd*100 + d*100 + r-10
            dlo = max(c0 // 100, d0)
            dhi = c1 // 100
            o = acc_h.rearrange("p (d r) -> p d r", r=100)[
                :, dlo - c0 // 100:dhi - c0 // 100, 10:100]
            ib = bb + Dd * 100 + dlo * 100
            i = xf[:, ib:ib + (dhi - dlo) * 100].rearrange(
                "p (d r) -> p d r", r=100)[:, :, 0:90]
            ops.append((o, i, t))
        else:
            # dh==-1 and dw==-1: extended full-block op + 2 negated comps
            s = bb + Dd * 100 - 11
            a0 = max(c0, d0 * 100)
            ops.append((acc_h[:, a0 - c0:c1 - c0], xf[:, a0 + s:c1 + s], t))
            dlo = max(c0 // 100, d0)
            dhi = c1 // 100
            ov = acc_h.rearrange("p (d q w) -> p d q w", q=10, w=10)
            iv = xf[:, dlo * 100 + s:dhi * 100 + s].rearrange(
                "p (d q w) -> p d q w", q=10, w=10)
            dsl = slice(dlo - c0 // 100, dhi - c0 // 100)
            # comp1: out (d, q full, w=0) wrongly read (d, q-1, 9)
            ops.append((ov[:, dsl, :, 0], iv[:, :, :, 0], _NEG[t]))
            # comp2: out (d, q=0, w 1..9) wrongly read (d-1, 9, w-1)
            ops.append((ov[:, dsl, 0, 1:10], iv[:, :, 0, 1:10], _NEG[t]))
    return ops


def _ln_stats(nc, work, ones1_sb, ones128_sb, x_tiles, sq_tiles, width,
              eps_sb, sx, sxx, muB, rstdB):
    """Cross-partition LayerNorm stats for C=512 split over 4 partition tiles.

    x_tiles/sq_tiles: lists of 4 fp16 APs, each (128, width). sx/sxx are
    (1, width) f32 PSUM APs; muB/rstdB are (128, width) f32 PSUM APs that
    receive the broadcast mean / inverse-std.
    """
    f32, f16 = dt.float32, dt.float16
    n = len(x_tiles)
    for i, xt in enumerate(x_tiles):
        nc.tensor.matmul(sx, ones1_sb[:], xt, start=(i == 0), stop=(i == n - 1))
    for i, st in enumerate(sq_tiles):
        nc.tensor.matmul(sxx, ones1_sb[:], st, start=(i == 0), stop=(i == n - 1))
    mu_r = work.tile([1, width], f16, tag="mu", bufs=2)
    nc.vector.tensor_scalar_mul(out=mu_r[:], in0=sx, scalar1=1.0 / C)
    msq = work.tile([1, width], f32, tag="msq", bufs=2)
    nc.vector.tensor_scalar_mul(out=msq[:], in0=sxx, scalar1=1.0 / C)
    mu2 = work.tile([1, width], f32, tag="mu2", bufs=2)
    nc.vector.tensor_mul(out=mu2[:], in0=mu_r[:], in1=mu_r[:])
    var = work.tile([1, width], f32, tag="var", bufs=2)
    nc.vector.tensor_sub(out=var[:], in0=msq[:], in1=mu2[:])
    std = work.tile([1, width], f32, tag="std", bufs=2)
    nc.scalar.activation(std[:], var[:], Act.Sqrt, bias=eps_sb[0:1, 0:1])
    rstd_r = work.tile([1, width], f16, tag="rstd", bufs=2)
    nc.vector.reciprocal(out=rstd_r[:], in_=std[:])
    nc.tensor.matmul(muB, ones128_sb[:], mu_r[:], start=True, stop=True)
    nc.tensor.matmul(rstdB, ones128_sb[:], rstd_r[:], start=True, stop=True)


def _build_program():
    nc = bacc.Bacc("TRN2", target_bir_lowering=False, debug=False, num_devices=8)
    f32, f16 = dt.float32, dt.float16

    xqtd = nc.dram_tensor("xqt", [CT, P, NCHUNK], f16, kind="ExternalInput").ap()
    xtd = nc.dram_tensor("xtd", [CT, P, N], f16, kind="ExternalInput").ap()
    wq = nc.dram_tensor("wq", [C, C], f16, kind="ExternalInput").ap()
    wkv = nc.dram_tensor("wkv", [C, 2 * C], f16, kind="ExternalInput").ap()
    wp = nc.dram_tensor("wp", [C, C], f16, kind="ExternalInput").ap()
    wdiagd = nc.dram_tensor("wdiag", [CT, NTAP, P, P], f16, kind="ExternalInput").ap()
    vecsd = nc.dram_tensor("vecs", [C, 7], f32, kind="ExternalInput").ap()
    bkvd = nc.dram_tensor("bkv", [2 * C], f32, kind="ExternalInput").ap()
    utd = nc.dram_tensor("ut", [NSR, NCHUNK], f16, kind="ExternalInput").ap()
    eyed = nc.dram_tensor("eye", [P, P], f16, kind="ExternalInput").ap()
    e8d = nc.dram_tensor("e8", [H, C], f16, kind="ExternalInput").ap()
    ones1d = nc.dram_tensor("ones1", [P, 1], f16, kind="ExternalInput").ap()
    ones128d = nc.dram_tensor("ones128", [1, P], f16, kind="ExternalInput").ap()
    epsd = nc.dram_tensor("epsv", [P, 1], f32, kind="ExternalInput").ap()
    yt = nc.dram_tensor("yt", [C, NCHUNK], f16, kind="ExternalOutput").ap()

    with TileContext(nc) as tc, nc.allow_low_precision(
            reason="fp16 data with fp32 PSUM accumulation; tol is 2e-2"):
        with contextlib.ExitStack() as octx:
            consts = octx.enter_context(tc.tile_pool(name="consts", bufs=1))
            keep1 = octx.enter_context(tc.tile_pool(name="keep1", bufs=1))
            work = octx.enter_context(tc.tile_pool(name="work", bufs=2))

            # ---------- constants ----------
            eye_sb = consts.tile([P, P], f16)
            nc.sync.dma_start(out=eye_sb[:], in_=eyed[:])
            e8_sb = consts.tile([H, C], f16)
            nc.sync.dma_start(out=e8_sb[:], in_=e8d[:])
            ones1_sb = consts.tile([P, 1], f16)
            nc.sync.dma_start(out=ones1_sb[:], in_=ones1d[:])
            ones128_sb = consts.tile([1, P], f16)
            nc.sync.dma_start(out=ones128_sb[:], in_=ones128d[:])
            eps_sb = consts.tile([P, 1], f32)
            nc.sync.dma_start(out=eps_sb[:], in_=epsd[:])
            vecs_sb = consts.tile([P, CT, 7], f32)
            nc.sync.dma_start(out=vecs_sb[:], in_=vecsd.rearrange("(o p) t -> p o t", p=P))
            bkv_sb = consts.tile([P, 2 * CT], f32)
            nc.sync.dma_start(out=bkv_sb[:], in_=bkvd.rearrange("(o p) -> p o", p=P))

            qT = keep1.tile([P, CT, NCHUNK], f16)       # 16 KB/part
            kT = keep1.tile([P, CT, NSR], f16)          # 8 KB/part
            lnidT = keep1.tile([P, CT, NCHUNK], f16)    # 16 KB/part
            wp_sb = keep1.tile([P, CT, C], f16)
            v_nat = keep1.tile([P, MT, C], f16)
            v_aug = keep1.tile([P, MT, H, HD + 1], f16)

            # ================= phase 1: conv + LN + kv + v ==================
            with contextlib.ExitStack() as ectx:
                psE = ectx.enter_context(tc.tile_pool(name="psE", bufs=1, space="PSUM"))

                with tc.tile_pool(name="cpool", bufs=1) as cpool, \
                        tc.tile_pool(name="dgp", bufs=1) as dgp:
                    diag_sb = dgp.tile([P, CT, NTAP, P], f16)   # 30 KB/part
                    nc.sync.dma_start(
                        out=diag_sb[:],
                        in_=wdiagd.rearrange("c t k m -> k c t m"))
                    wkv_sb = dgp.tile([P, CT, 2 * C], f16)
                    nc.sync.dma_start(out=wkv_sb[:],
                                      in_=wkv.rearrange("(k p) m -> p k m", p=P))
                    nc.sync.dma_start(out=wp_sb[:],
                                      in_=wp.rearrange("(k p) m -> p k m", p=P))

                    # ---------- conv on the PE, evac+bias+square on scalar ---
                    xr = cpool.tile([P, CT, NSR], f16)
                    sq = cpool.tile([P, CT, NSR], f16)
                    for ct in range(CT):
                        xct = cpool.tile([P, N], f16, tag="xct", bufs=2)
                        nc.sync.dma_start(out=xct[:], in_=xtd[ct, :, :])
                        acc = psE.tile([P, 2, BW], f32, tag="e", bufs=3)
                        for hf in range(2):
                            ops = _conv_ops(xct[:], acc[:, hf, 0:RCW], hf)
                            for i, (o, inp, t) in enumerate(ops):
                                nc.tensor.matmul(
                                    o, diag_sb[:, ct, t, :], inp,
                                    start=(i == 0), stop=(i == len(ops) - 1))
                        cs = slice(0, NSR)
                        nc.scalar.activation(xr[:, ct, :], acc[:, :, 0:RCW],
                                             Act.Identity, bias=vecs_sb[:, ct, 1:2])
                        nc.scalar.activation(sq[:, ct, :], acc[:, :, 0:RCW],
                                             Act.Square, bias=vecs_sb[:, ct, 1:2])

                    # ---------- LayerNorm over C -> xrn ----------
                    xrn = cpool.tile([P, CT, NSR], f16)
                    for ch in range(2):
                        cs = slice(ch * RCW, (ch + 1) * RCW)
                        st = psE.tile([P, 2, BW], f32, tag="e", bufs=3)
                        bc = psE.tile([P, 2, BW], f32, tag="e", bufs=3)
                        _ln_stats(nc, work, ones1_sb, ones128_sb,
                                  [xr[:, ct, cs] for ct in range(CT)],
                                  [sq[:, ct, cs] for ct in range(CT)], RCW,
                                  eps_sb, st[0:1, 0, 0:RCW], st[0:1, 1, 0:RCW],
                                  bc[:, 0, 0:RCW], bc[:, 1, 0:RCW])
                        for ct in range(CT):
                            t1 = work.tile([P, RCW], f32, tag="lnt")
                            nc.vector.tensor_sub(out=t1[:], in0=xr[:, ct, cs],
                                                 in1=bc[:, 0, 0:RCW])
                            t2 = work.tile([P, RCW], f32, tag="lnt2")
                            nc.vector.tensor_mul(out=t2[:], in0=t1[:],
                                                 in1=bc[:, 1, 0:RCW])
                            nc.vector.tensor_scalar(
                                out=xrn[:, ct, cs], in0=t2[:],
                                scalar1=vecs_sb[:, ct, 2:3], scalar2=vecs_sb[:, ct, 3:4],
                                op0=Alu.mult, op1=Alu.add)

                    # ---------- kv projection ----------
                    vT = cpool.tile([P, CT, NSR], f16)
                    for mt8 in range(2 * CT):
                        dsts = kT if mt8 < CT else vT
                        di = mt8 if mt8 < CT else mt8 - CT
                        acc = psE.tile([P, 2, BW], f32, tag="e", bufs=3)
                        for ch in range(2):
                            cs = slice(ch * RCW, (ch + 1) * RCW)
                            for kt in range(CT):
                                nc.tensor.matmul(
                                    acc[:, ch, 0:RCW],
                                    wkv_sb[:, kt, mt8 * P:(mt8 + 1) * P],
                                    xrn[:, kt, cs],
                                    start=(kt == 0), stop=(kt == CT - 1))
                        nc.vector.tensor_scalar_add(
                            out=dsts[:, di, :],
                            in0=acc[:, :, 0:RCW],
                            scalar1=bkv_sb[:, mt8:mt8 + 1])

                    # ---------- v natural + ones column (v_aug) ----------
                    nc.gpsimd.tensor_copy(
                        out=v_aug[:, :, :, HD:HD + 1],
                        in_=ones1_sb[:, 0:1, None, None].to_broadcast([P, MT, H, 1]))
                    for ci in range(CT):
                        for mt in range(MT):
                            tp = psE.tile([P, P], f16, tag="trps", bufs=2)
                            nc.tensor.transpose(
                                tp[:MTW, :], vT[:, ci, mt * MTW:(mt + 1) * MTW],
                                eye_sb[:])
                            nc.vector.tensor_copy(
                                out=v_nat[:MTW, mt, ci * P:(ci + 1) * P],
                                in_=tp[:MTW, :])
                            nc.gpsimd.tensor_copy(
                                out=v_aug[:MTW, mt, 2 * ci, 0:HD],
                                in_=v_nat[:MTW, mt, ci * P:ci * P + HD])
                            nc.gpsimd.tensor_copy(
                                out=v_aug[:MTW, mt, 2 * ci + 1, 0:HD],
                                in_=v_nat[:MTW, mt, ci * P + HD:(ci + 1) * P])

            # ============ phases 2+3: q proj, identity, attention ===========
            with contextlib.ExitStack() as actx:
                psA = actx.enter_context(tc.tile_pool(name="psA", bufs=1, space="PSUM"))
                ld2 = actx.enter_context(tc.tile_pool(name="ld2", bufs=2))
                ppool = actx.enter_context(tc.tile_pool(name="ppool", bufs=2))
                apool = actx.enter_context(tc.tile_pool(name="apool", bufs=1))

                # ---------- phase 2a: q projection ----------
                with tc.tile_pool(name="wqp", bufs=1) as wqp:
                    wq_sb = wqp.tile([P, CT, C], f16)
                    nc.sync.dma_start(out=wq_sb[:],
                                      in_=wq.rearrange("(k p) m -> p k m", p=P))
                    xqT = wqp.tile([P, CT, NCHUNK], f16)
                    for ct in range(CT):
                        nc.sync.dma_start(out=xqT[:, ct, :], in_=xqtd[ct, :, :])
                    for ct in range(CT):
                        for rc in range(RC):
                            acc = psA.tile([P, BW], f32, tag="ov", bufs=4)
                            for kt in range(CT):
                                nc.tensor.matmul(
                                    acc[:, 0:RCW], wq_sb[:, kt, ct * P:(ct + 1) * P],
                                    xqT[:, kt, rc * RCW:(rc + 1) * RCW],
                                    start=(kt == 0), stop=(kt == CT - 1))
                            nc.scalar.activation(
                                qT[:, ct, rc * RCW:(rc + 1) * RCW], acc[:, 0:RCW],
                                Act.Identity, bias=vecs_sb[:, ct, 0:1])

                    # ---------- phase 2b: identity branch (U matmul) + LN ---
                    for rc in range(RC):
                        rs = slice(rc * RCW, (rc + 1) * RCW)
                        ut_t = []
                        for mt in range(MT):
                            u1 = ld2.tile([P, RCW], f16, tag="uld", bufs=10)
                            nc.sync.dma_start(out=u1[:MTW, :],
                                              in_=utd[mt * MTW:(mt + 1) * MTW, rs])
                            ut_t.append(u1)
                        idp = psA.tile([P, 4, BW], f32, tag="sc", bufs=1)
                        for ct in range(CT):
                            for mt in range(MT):
                                nc.tensor.matmul(
                                    idp[:, ct, 0:RCW],
                                    v_nat[:MTW, mt, ct * P:(ct + 1) * P],
                                    ut_t[mt][:MTW, :],
                                    start=(mt == 0), stop=(mt == MT - 1))
                        idr = apool.tile([P, CT, RCW], f16, tag="idr", bufs=2)
                        idsq = apool.tile([P, CT, RCW], f16, tag="idsq", bufs=2)
                        nc.vector.tensor_copy(out=idr[:], in_=idp[:, :, 0:RCW])
                        nc.scalar.activation(idsq[:], idp[:, :, 0:RCW], Act.Square)
                        st1 = psA.tile([P, BW], f32, tag="ov", bufs=4)
                        st2 = psA.tile([P, BW], f32, tag="ov", bufs=4)
                        bc = psA.tile([P, BW], f32, tag="ov", bufs=4)
                        bc2 = psA.tile([P, BW], f32, tag="ov", bufs=4)
                        _ln_stats(nc, work, ones1_sb, ones128_sb,
                                  [idr[:, ct, :] for ct in range(CT)],
                                  [idsq[:, ct, :] for ct in range(CT)], RCW,
                                  eps_sb, st1[0:1, 0:RCW], st2[0:1, 0:RCW],
                                  bc[:, 0:RCW], bc2[:, 0:RCW])
                        for ct in range(CT):
                            t1 = work.tile([P, RCW], f32, tag="lnt")
                            nc.vector.tensor_sub(out=t1[:], in0=idr[:, ct, :],
                                                 in1=bc[:, 0:RCW])
                            t2 = work.tile([P, RCW], f32, tag="lnt2")
                            nc.vector.tensor_mul(out=t2[:], in0=t1[:],
                                                 in1=bc2[:, 0:RCW])
                            nc.vector.tensor_scalar(
                                out=lnidT[:, ct, rs], in0=t2[:],
                                scalar1=vecs_sb[:, ct, 4:5], scalar2=vecs_sb[:, ct, 5:6],
                                op0=Alu.mult, op1=Alu.add)

                # ---------- phase 3: attention (scalar engine = Exp only) ---
                for rc in range(RC):
                    rs = slice(rc * RCW, (rc + 1) * RCW)
                    oT65 = apool.tile([P, H, RCW], f16, tag="ot65", bufs=2)
                    for hh in range(H):
                        pb = HD * (hh % 2)
                        ci = hh // 2
                        pT = ppool.tile([P, MT, BW], f16, tag="pt")
                        ov = psA.tile([P, BW], f32, tag="ov", bufs=4)
                        for g in range(2):
                            sc = psA.tile([P, 4, BW], f32, tag="sc", bufs=1)
                            for k in range(4):
                                mt = 4 * g + k
                                nc.tensor.matmul(
                                    sc[:MTW, k, 0:RCW],
                                    kT[pb:pb + HD, ci, mt * MTW:(mt + 1) * MTW],
                                    qT[pb:pb + HD, ci, rs],
                                    start=True, stop=True)
                            nc.scalar.activation(pT[:MTW, 4 * g:4 * g + 4, :],
                                                 sc[:MTW, :, :], Act.Exp,
                                                 scale=SCALE)
                        for mt in range(MT):
                            nc.tensor.matmul(
                                ov[0:HD + 1, 0:RCW], v_aug[:MTW, mt, hh, :],
                                pT[:MTW, mt, 0:RCW],
                                start=(mt == 0), stop=(mt == MT - 1))
                        nc.vector.tensor_copy(out=oT65[0:HD + 1, hh, :],
                                              in_=ov[0:HD + 1, 0:RCW])

                    # --- normalize + add identity + output projection ---
                    den8 = apool.tile([H, RCW], f16, tag="den8", bufs=2)
                    for hh in range(H):
                        nc.sync.dma_start(out=den8[hh:hh + 1, :],
                                          in_=oT65[HD:HD + 1, hh, :])
                    rec8 = apool.tile([H, RCW], f16, tag="rec8", bufs=2)
                    nc.vector.reciprocal(out=rec8[:], in_=den8[:])
                    sum_r = apool.tile([P, CT, RCW], f16, tag="sumr", bufs=2)
                    for ct in range(CT):
                        recB = psA.tile([P, BW], f32, tag="ov", bufs=4)
                        nc.tensor.matmul(recB[:, 0:RCW],
                                         e8_sb[:, ct * P:(ct + 1) * P],
                                         rec8[:], start=True, stop=True)
                        tmp = work.tile([P, RCW], f32, tag="ntmp")
                        nc.vector.tensor_mul(out=tmp[0:HD, :],
                                             in0=oT65[0:HD, 2 * ct, :],
                                             in1=recB[0:HD, 0:RCW])
                        nc.vector.tensor_mul(out=tmp[HD:P, :],
                                             in0=oT65[0:HD, 2 * ct + 1, :],
                                             in1=recB[HD:P, 0:RCW])
                        nc.vector.tensor_add(out=sum_r[:, ct, :], in0=tmp[:],
                                             in1=lnidT[:, ct, rs])
                    for ct2 in range(CT):
                        fin = psA.tile([P, BW], f32, tag="ov", bufs=4)
                        for kt in range(CT):
                            nc.tensor.matmul(
                                fin[:, 0:RCW],
                                wp_sb[:, kt, ct2 * P:(ct2 + 1) * P],
                                sum_r[:, kt, :],
                                start=(kt == 0), stop=(kt == CT - 1))
                        oF = apool.tile([P, RCW], f16, tag="of", bufs=2)
                        nc.vector.tensor_scalar_add(out=oF[:], in0=fin[:, 0:RCW],
                                                    scalar1=vecs_sb[:, ct2, 6:7])
                        nc.sync.dma_start(out=yt[ct2 * P:(ct2 + 1) * P, rs], in_=oF[:])

    nc.finalize()
    return nc


def _parity_perm():
    perm = np.empty(N, np.int64)
    for a in range(2):
        for b in range(2):
            for c in range(2):
                blk = (a * 4 + b * 2 + c) * NSR
                for d in range(DR):
                    for h in range(DR):
                        for w_ in range(DR):
                            perm[blk + d * 100 + h * 10 + w_] = (
                                (2 * d + a) * 400 + (2 * h + b) * 20 + (2 * w_ + c))
    return perm


def _host_consts():
    eye = np.eye(P, dtype=np.float16)
    e8 = np.zeros((H, C), np.float16)
    for p in range(C):
        hh = 2 * (p // P) + (p % P) // HD
        e8[hh, p] = 1.0
    ones1 = np.ones((P, 1), np.float16)
    ones128 = np.ones((1, P), np.float16)
    epsv = np.full((P, 1), EPS, np.float32)
    return eye, e8, ones1, ones128, epsv


def _interp_1d(n_out, n_in, off):
    out = []
    for i in range(n_out):
        src = (off + i + 0.5) / 2.0 - 0.5
        lo = int(np.floor(src))
        f = src - lo
        lo_c = min(max(lo, 0), n_in - 1)
        hi_c = min(max(lo + 1, 0), n_in - 1)
        out.append(((lo_c, 1.0 - f), (hi_c, f)))
    return out


def _build_ut(j):
    """U^T (NSR, NCHUNK): idT[:, n] = sum_m v_nat[m, :] * UT[m, n], quarter j."""
    ut = np.zeros((NSR, NCHUNK), np.float32)
    d_lo = (j * NCHUNK) // (D3 * D3)
    dmap = _interp_1d(5, DR, d_lo)
    hmap = _interp_1d(D3, DR, 0)
    wmap = _interp_1d(D3, DR, 0)
    for dd in range(5):
        for hh2 in range(D3):
            for ww in range(D3):
                nloc = dd * D3 * D3 + hh2 * D3 + ww
                for (di, dwt) in dmap[dd]:
                    for (hi, hwt) in hmap[hh2]:
                        for (wi, wwt) in wmap[ww]:
                            m = di * DR * DR + hi * DR + wi
                            ut[m, nloc] += dwt * hwt * wwt
    return ut.astype(np.float16)


def _build_wdiag(sr_w):
    """Per-tap diagonal weight matrices: [CT, NTAP, P, P] fp16.

    Slots 0..26 are the 27 conv taps; slots 27..29 are the negated weights
    of the three dh==-1,dw==-1 taps (t = 0, 9, 18) for wrap compensation.
    """
    w27 = sr_w.reshape(C, 27).astype(np.float32)
    d = np.zeros((CT, NTAP, P, P), np.float16)
    idx = np.arange(P)
    for ct in range(CT):
        blk = w27[ct * P:(ct + 1) * P]        # (P, 27)
        d[ct, :27, idx, idx] = blk.astype(np.float16)
        for t, slot in _NEG.items():
            d[ct, slot, idx, idx] = (-blk[:, t]).astype(np.float16)
    return np.ascontiguousarray(d)


def kernel(**inputs):
    global _PROGRAM, _HOST, LAST_RESULT
    x = np.asarray(inputs["x"], np.float32)
    Wq = np.asarray(inputs["Wq"], np.float32)
    bq = np.asarray(inputs["bq"], np.float32)
    Wkv = np.asarray(inputs["Wkv"], np.float32)
    bkv_ = np.asarray(inputs["bkv"], np.float32)
    sr_w = np.asarray(inputs["sr_w"], np.float32)
    sr_b = np.asarray(inputs["sr_b"], np.float32)
    sr_g = np.asarray(inputs["sr_g"], np.float32)
    sr_beta = np.asarray(inputs["sr_beta"], np.float32)
    up_g = np.asarray(inputs["up_g"], np.float32)
    up_beta = np.asarray(inputs["up_beta"], np.float32)
    Wp = np.asarray(inputs["Wp"], np.float32)
    bp = np.asarray(inputs["bp"], np.float32)

    if _PROGRAM is None:
        _PROGRAM = _build_program()
    nc = _PROGRAM

    if _HOST is None:
        _HOST = (_host_consts(), [_build_ut(j) for j in range(4)], _parity_perm())
    (eye, e8, ones1, ones128, epsv), uts, perm = _HOST

    wdiag = _build_wdiag(sr_w)
    vecs = np.ascontiguousarray(
        np.stack([bq, sr_b, sr_g, sr_beta, up_g, up_beta, bp], axis=1))
    wq16 = np.ascontiguousarray(Wq.astype(np.float16))
    wkv16 = np.ascontiguousarray(Wkv.astype(np.float16))
    wp16 = np.ascontiguousarray(Wp.astype(np.float16))

    xtds, xqts = [], []
    for b in range(B):
        xtds.append(np.ascontiguousarray(
            x[b][perm].T.reshape(CT, P, N).astype(np.float16)))
        xqts.append([np.ascontiguousarray(
            x[b, j * NCHUNK:(j + 1) * NCHUNK].T.reshape(CT, P, NCHUNK)
            .astype(np.float16)) for j in range(4)])

    in_maps = []
    for core in range(8):
        b, j = core // 4, core % 4
        in_maps.append({
            "xtd": xtds[b],
            "xqt": xqts[b][j],
            "wq": wq16, "wkv": wkv16, "wp": wp16,
            "wdiag": wdiag, "vecs": vecs, "bkv": bkv_,
            "ut": uts[j],
            "eye": eye, "e8": e8, "ones1": ones1, "ones128": ones128,
            "epsv": epsv,
        })

    res = run_bass_kernel_spmd(nc, in_maps, core_ids=list(range(8)), trace=TRACE)
    LAST_RESULT = res
    out = np.empty((B, N, C), np.float32)
    for core in range(8):
        b, j = core // 4, core % 4
        out[b, j * NCHUNK:(j + 1) * NCHUNK, :] = (
            res.results[core]["yt"].astype(np.float32).T)
    return out


# revision 15
# speedup vs baseline: 1.0815x; 1.0815x over previous
"""Sparse-attention (PVT-style SRA) kernel for 8 Trainium2 NeuronCores.

Sharding: 8 cores = 2 batches x 4 row-quarters of N=8000. Each core computes
its 2000 output rows end-to-end; the spatial-reduction branch (conv+LN+kv) is
replicated per batch. All matmuls run in fp16 (1 cycle/row on the PE, fast
weight load); accumulation stays fp32 in PSUM. The depthwise conv runs on the
tensor engine as diagonal-weight matmuls accumulating all 27 taps in PSUM;
the trilinear upsample is an interpolation matmul whose weights (products of
1/4 and 3/4) are fp16-exact. Host pre-transposes x into channel-major layout
so the kernel does no PE transposes of the input.

Phases: (1) conv + LN + kv + v transposes, (2) q projection + upsampled
identity branch + LN, (3) attention with an Exp-only scalar engine, then
normalize + output projection per row chunk.
"""

import sys

sys.path.insert(0, "/opt/trn_rl_repo")

import contextlib
import numpy as np
import concourse.bacc as bacc
import concourse.mybir as mybir
from concourse.tile import TileContext
from concourse.bass_utils import run_bass_kernel_spmd

dt = mybir.dt
Alu = mybir.AluOpType
Act = mybir.ActivationFunctionType

P = 128
B, N, C = 2, 8000, 512
H, HD = 8, 64
D3 = 20          # full spatial edge (D=H=W)
DR = 10          # reduced spatial edge
NSR = 1000       # DR**3
CT = 4           # C // P
NCHUNK = 2000    # output rows per core
RC = 4           # row chunks per core
RCW = 500        # rows per chunk
MT = 8           # key tiles
MTW = 125        # keys per tile
BW = 512         # PSUM bank width (f32 elems)
NTAP = 30        # 27 taps + 3 negated wrap-compensation taps
SCALE = HD ** -0.5
EPS = 1e-6

# dh==-1 and dw==-1 taps get wrap compensation with negated weights at
# diag slots 27..29 (tap index t -> neg slot).
_NEG = {0: 27, 9: 28, 18: 29}

_PROGRAM = None
_HOST = None
TRACE = False
LAST_RESULT = None


def _conv_ops(xf, acc_h, h):
    """Matmul op list for one 500-col half of the conv output block.

    The host permutes x's spatial rows into parity-block order: flat index
    (a*4+b*2+c)*1000 + d*100 + h*10 + w  <->  original (2d+a, 2h+b, 2w+c).
    Every tap then reduces to a shifted access pattern on xf, and the conv
    becomes a sum over taps of diag(w_tap) @ x_shifted, accumulated in PSUM.
    The center tap comes first and covers the whole half, so it owns
    start=True. Returns [(out_ap, in_ap, diag_slot)].

    xf: (p, 8000) fp16 input view; acc_h: (p, 500) f32 PSUM AP for output
    columns [h*500, (h+1)*500) of the 1000-col reduced block.
    """
    c0 = h * RCW
    c1 = c0 + RCW
    ops = []
    taps = [(0, 0, 0)] + [(a, b, c)
                          for a in (-1, 0, 1) for b in (-1, 0, 1) for c in (-1, 0, 1)
                          if (a, b, c) != (0, 0, 0)]
    for (dd, dh, dw) in taps:
        pa, pb, pc = (0 if dd == 0 else 1), (0 if dh == 0 else 1), (0 if dw == 0 else 1)
        Dd, Dh = (-1 if dd == -1 else 0), (-1 if dh == -1 else 0)
        bb = (pa * 4 + pb * 2 + pc) * 1000
        d0 = 1 if dd == -1 else 0
        t = (dd + 1) * 9 + (dh + 1) * 3 + (dw + 1)
        if dh != -1 and dw != -1:
            # contiguous: out o gets in bb + Dd*100 + o, o in [d0*100, 1000)
            a0 = max(c0, d0 * 100)
            ib = bb + Dd * 100 + a0
            ops.append((acc_h[:, a0 - c0:c1 - c0], xf[:, ib:ib + (c1 - a0)], t))
        elif dw == -1 and dh != -1:
            # out (x, w in 1..10), x = d*10+h_ in [d0*10, 100);
            # in = bb + Dd*100 + x*10 + (w-1)
            x0 = max(c0 // 10, d0 * 10)
            x1 = c1 // 10
            o = acc_h.rearrange("p (x w) -> p x w", w=10)[
                :, x0 - c0 // 10:x1 - c0 // 10, 1:10]
            ib = bb + Dd * 100 + x0 * 10
            i = xf[:, ib:ib + (x1 - x0) * 10].rearrange(
                "p (x w) -> p x w", w=10)[:, :, 0:9]
            ops.append((o, i, t))
        elif dh == -1 and dw != -1:
            # out (d, r in 10..100), d in [d0, 10); in = bb + Dd*100 + d*100 + r-10
            dlo = max(c0 // 100, d0)
            dhi = c1 // 100
            o = acc_h.rearrange("p (d r) -> p d r", r=100)[
                :, dlo - c0 // 100:dhi - c0 // 100, 10:100]
            ib = bb + Dd * 100 + dlo * 100
            i = xf[:, ib:ib + (dhi - dlo) * 100].rearrange(
                "p (d r) -> p d r", r=100)[:, :, 0:90]
            ops.append((o, i, t))
        else:
            # dh==-1 and dw==-1: extended full-block op + 2 negated comps
            s = bb + Dd * 100 - 11
            a0 = max(c0, d0 * 100)
            ops.append((acc_h[:, a0 - c0:c1 - c0], xf[:, a0 + s:c1 + s], t))
            dlo = max(c0 // 100, d0)
            dhi = c1 // 100
            ov = acc_h.rearrange("p (d q w) -> p d q w", q=10, w=10)
            iv = xf[:, dlo * 100 + s:dhi * 100 + s].rearrange(
                "p (d q w) -> p d q w", q=10, w=10)
            dsl = slice(dlo - c0 // 100, dhi - c0 // 100)
            # comp1: out (d, q full, w=0) wrongly read (d, q-1, 9)
            ops.append((ov[:, dsl, :, 0], iv[:, :, :, 0], _NEG[t]))
            # comp2: out (d, q=0, w 1..9) wrongly read (d-1, 9, w-1)
            ops.append((ov[:, dsl, 0, 1:10], iv[:, :, 0, 1:10], _NEG[t]))
    return ops


def _ln_stats(nc, work, ones1_sb, ones128_sb, x_tiles, sq_tiles, width,
              eps_sb, sx, sxx, muB, rstdB):
    """Cross-partition LayerNorm stats for C=512 split over 4 partition tiles.

    x_tiles/sq_tiles: lists of 4 fp16 APs, each (128, width). sx/sxx are
    (1, width) f32 PSUM APs; muB/rstdB are (128, width) f32 PSUM APs that
    receive the broadcast mean / inverse-std.
    """
    f32, f16 = dt.float32, dt.float16
    n = len(x_tiles)
    for i, xt in enumerate(x_tiles):
        nc.tensor.matmul(sx, ones1_sb[:], xt, start=(i == 0), stop=(i == n - 1))
    for i, st in enumerate(sq_tiles):
        nc.tensor.matmul(sxx, ones1_sb[:], st, start=(i == 0), stop=(i == n - 1))
    mu_r = work.tile([1, width], f16, tag="mu", bufs=2)
    nc.vector.tensor_scalar_mul(out=mu_r[:], in0=sx, scalar1=1.0 / C)
    msq = work.tile([1, width], f32, tag="msq", bufs=2)
    nc.vector.tensor_scalar_mul(out=msq[:], in0=sxx, scalar1=1.0 / C)
    mu2 = work.tile([1, width], f32, tag="mu2", bufs=2)
    nc.vector.tensor_mul(out=mu2[:], in0=mu_r[:], in1=mu_r[:])
    var = work.tile([1, width], f32, tag="var", bufs=2)
    nc.vector.tensor_sub(out=var[:], in0=msq[:], in1=mu2[:])
    std = work.tile([1, width], f32, tag="std", bufs=2)
    nc.scalar.activation(std[:], var[:], Act.Sqrt, bias=eps_sb[0:1, 0:1])
    rstd_r = work.tile([1, width], f16, tag="rstd", bufs=2)
    nc.vector.reciprocal(out=rstd_r[:], in_=std[:])
    nc.tensor.matmul(muB, ones128_sb[:], mu_r[:], start=True, stop=True)
    nc.tensor.matmul(rstdB, ones128_sb[:], rstd_r[:], start=True, stop=True)


def _build_program():
    nc = bacc.Bacc("TRN2", target_bir_lowering=False, debug=False, num_devices=8)
    f32, f16 = dt.float32, dt.float16

    xqtd = nc.dram_tensor("xqt", [CT, P, NCHUNK], f16, kind="ExternalInput").ap()
    xtd = nc.dram_tensor("xtd", [CT, P, N], f16, kind="ExternalInput").ap()
    wq = nc.dram_tensor("wq", [C, C], f16, kind="ExternalInput").ap()
    wkv = nc.dram_tensor("wkv", [C, 2 * C], f16, kind="ExternalInput").ap()
    wp = nc.dram_tensor("wp", [C, C], f16, kind="ExternalInput").ap()
    wdiagd = nc.dram_tensor("wdiag", [CT, NTAP, P, P], f16, kind="ExternalInput").ap()
    vecsd = nc.dram_tensor("vecs", [C, 7], f32, kind="ExternalInput").ap()
    bkvd = nc.dram_tensor("bkv", [2 * C], f32, kind="ExternalInput").ap()
    utd = nc.dram_tensor("ut", [NSR, NCHUNK], f16, kind="ExternalInput").ap()
    eyed = nc.dram_tensor("eye", [P, P], f16, kind="ExternalInput").ap()
    e8d = nc.dram_tensor("e8", [H, C], f16, kind="ExternalInput").ap()
    ones1d = nc.dram_tensor("ones1", [P, 1], f16, kind="ExternalInput").ap()
    ones128d = nc.dram_tensor("ones128", [1, P], f16, kind="ExternalInput").ap()
    epsd = nc.dram_tensor("epsv", [P, 1], f32, kind="ExternalInput").ap()
    yt = nc.dram_tensor("yt", [C, NCHUNK], f16, kind="ExternalOutput").ap()

    with TileContext(nc) as tc, nc.allow_low_precision(
            reason="fp16 data with fp32 PSUM accumulation; tol is 2e-2"):
        with contextlib.ExitStack() as octx:
            consts = octx.enter_context(tc.tile_pool(name="consts", bufs=1))
            keep1 = octx.enter_context(tc.tile_pool(name="keep1", bufs=1))
            work = octx.enter_context(tc.tile_pool(name="work", bufs=2))

            # ---------- constants ----------
            eye_sb = consts.tile([P, P], f16)
            nc.sync.dma_start(out=eye_sb[:], in_=eyed[:])
            e8_sb = consts.tile([H, C], f16)
            nc.sync.dma_start(out=e8_sb[:], in_=e8d[:])
            ones1_sb = consts.tile([P, 1], f16)
            nc.sync.dma_start(out=ones1_sb[:], in_=ones1d[:])
            ones128_sb = consts.tile([1, P], f16)
            nc.sync.dma_start(out=ones128_sb[:], in_=ones128d[:])
            eps_sb = consts.tile([P, 1], f32)
            nc.sync.dma_start(out=eps_sb[:], in_=epsd[:])
            vecs_sb = consts.tile([P, CT, 7], f32)
            nc.sync.dma_start(out=vecs_sb[:], in_=vecsd.rearrange("(o p) t -> p o t", p=P))
            bkv_sb = consts.tile([P, 2 * CT], f32)
            nc.sync.dma_start(out=bkv_sb[:], in_=bkvd.rearrange("(o p) -> p o", p=P))

            qT = keep1.tile([P, CT, NCHUNK], f16)       # 16 KB/part
            kT = keep1.tile([P, CT, NSR], f16)          # 8 KB/part
            lnidT = keep1.tile([P, CT, NCHUNK], f16)    # 16 KB/part
            wp_sb = keep1.tile([P, CT, C], f16)
            v_nat = keep1.tile([P, MT, C], f16)
            v_aug = keep1.tile([P, MT, H, HD + 1], f16)

            # ================= phase 1: conv + LN + kv + v ==================
            with contextlib.ExitStack() as ectx:
                psE = ectx.enter_context(tc.tile_pool(name="psE", bufs=1, space="PSUM"))

                with tc.tile_pool(name="cpool", bufs=1) as cpool, \
                        tc.tile_pool(name="dgp", bufs=1) as dgp:
                    diag_sb = dgp.tile([P, CT, NTAP, P], f16)   # 30 KB/part
                    nc.sync.dma_start(
                        out=diag_sb[:],
                        in_=wdiagd.rearrange("c t k m -> k c t m"))
                    wkv_sb = dgp.tile([P, CT, 2 * C], f16)
                    nc.sync.dma_start(out=wkv_sb[:],
                                      in_=wkv.rearrange("(k p) m -> p k m", p=P))
                    nc.sync.dma_start(out=wp_sb[:],
                                      in_=wp.rearrange("(k p) m -> p k m", p=P))

                    # ---------- conv on the PE, evac+bias+square on scalar ---
                    xr = cpool.tile([P, CT, NSR], f16)
                    sq = cpool.tile([P, CT, NSR], f16)
                    for ct in range(CT):
                        xct = cpool.tile([P, N], f16, tag="xct", bufs=2)
                        nc.sync.dma_start(out=xct[:], in_=xtd[ct, :, :])
                        acc = psE.tile([P, 2, BW], f32, tag="e", bufs=3)
                        for hf in range(2):
                            ops = _conv_ops(xct[:], acc[:, hf, 0:RCW], hf)
                            for i, (o, inp, t) in enumerate(ops):
                                nc.tensor.matmul(
                                    o, diag_sb[:, ct, t, :], inp,
                                    start=(i == 0), stop=(i == len(ops) - 1))
                        cs = slice(0, NSR)
                        nc.scalar.activation(xr[:, ct, :], acc[:, :, 0:RCW],
                                             Act.Identity, bias=vecs_sb[:, ct, 1:2])
                        nc.scalar.activation(sq[:, ct, :], acc[:, :, 0:RCW],
                                             Act.Square, bias=vecs_sb[:, ct, 1:2])

                    # ---------- LayerNorm over C -> xrn ----------
                    xrn = cpool.tile([P, CT, NSR], f16)
                    for ch in range(2):
                        cs = slice(ch * RCW, (ch + 1) * RCW)
                        st = psE.tile([P, 2, BW], f32, tag="e", bufs=3)
                        bc = psE.tile([P, 2, BW], f32, tag="e", bufs=3)
                        _ln_stats(nc, work, ones1_sb, ones128_sb,
                                  [xr[:, ct, cs] for ct in range(CT)],
                                  [sq[:, ct, cs] for ct in range(CT)], RCW,
                                  eps_sb, st[0:1, 0, 0:RCW], st[0:1, 1, 0:RCW],
                                  bc[:, 0, 0:RCW], bc[:, 1, 0:RCW])
                        for ct in range(CT):
                            t1 = work.tile([P, RCW], f32, tag="lnt")
                            nc.vector.tensor_sub(out=t1[:], in0=xr[:, ct, cs],
                                                 in1=bc[:, 0, 0:RCW])
                            t2 = work.tile([P, RCW], f32, tag="lnt2")
                            nc.vector.tensor_mul(out=t2[:], in0=t1[:],
                                                 in1=bc[:, 1, 0:RCW])
                            nc.vector.tensor_scalar(
                                out=xrn[:, ct, cs], in0=t2[:],
                                scalar1=vecs_sb[:, ct, 2:3], scalar2=vecs_sb[:, ct, 3:4],
                                op0=Alu.mult, op1=Alu.add)

                    # ---------- kv projection ----------
                    vT = cpool.tile([P, CT, NSR], f16)
                    for mt8 in range(2 * CT):
                        dsts = kT if mt8 < CT else vT
                        di = mt8 if mt8 < CT else mt8 - CT
                        acc = psE.tile([P, 2, BW], f32, tag="e", bufs=3)
                        for ch in range(2):
                            cs = slice(ch * RCW, (ch + 1) * RCW)
                            for kt in range(CT):
                                nc.tensor.matmul(
                                    acc[:, ch, 0:RCW],
                                    wkv_sb[:, kt, mt8 * P:(mt8 + 1) * P],
                                    xrn[:, kt, cs],
                                    start=(kt == 0), stop=(kt == CT - 1))
                        nc.vector.tensor_scalar_add(
                            out=dsts[:, di, :],
                            in0=acc[:, :, 0:RCW],
                            scalar1=bkv_sb[:, mt8:mt8 + 1])

                    # ---------- v natural + ones column (v_aug) ----------
                    nc.gpsimd.tensor_copy(
                        out=v_aug[:, :, :, HD:HD + 1],
                        in_=ones1_sb[:, 0:1, None, None].to_broadcast([P, MT, H, 1]))
                    for ci in range(CT):
                        for mt in range(MT):
                            tp = psE.tile([P, P], f16, tag="trps", bufs=2)
                            nc.tensor.transpose(
                                tp[:MTW, :], vT[:, ci, mt * MTW:(mt + 1) * MTW],
                                eye_sb[:])
                            nc.vector.tensor_copy(
                                out=v_nat[:MTW, mt, ci * P:(ci + 1) * P],
                                in_=tp[:MTW, :])
                            nc.gpsimd.tensor_copy(
                                out=v_aug[:MTW, mt, 2 * ci, 0:HD],
                                in_=v_nat[:MTW, mt, ci * P:ci * P + HD])
                            nc.gpsimd.tensor_copy(
                                out=v_aug[:MTW, mt, 2 * ci + 1, 0:HD],
                                in_=v_nat[:MTW, mt, ci * P + HD:(ci + 1) * P])

            # ============ phases 2+3: q proj, identity, attention ===========
            with contextlib.ExitStack() as actx:
                psA = actx.enter_context(tc.tile_pool(name="psA", bufs=1, space="PSUM"))
                ld2 = actx.enter_context(tc.tile_pool(name="ld2", bufs=2))
                ppool = actx.enter_context(tc.tile_pool(name="ppool", bufs=2))
                apool = actx.enter_context(tc.tile_pool(name="apool", bufs=1))

                # ---------- phase 2a: q projection ----------
                with tc.tile_pool(name="wqp", bufs=1) as wqp:
                    wq_sb = wqp.tile([P, CT, C], f16)
                    nc.sync.dma_start(out=wq_sb[:],
                                      in_=wq.rearrange("(k p) m -> p k m", p=P))
                    xqT = wqp.tile([P, CT, NCHUNK], f16)
                    for ct in range(CT):
                        nc.sync.dma_start(out=xqT[:, ct, :], in_=xqtd[ct, :, :])
                    for ct in range(CT):
                        for rc in range(RC):
                            acc = psA.tile([P, BW], f32, tag="ov", bufs=2)
                            for kt in range(CT):
                                nc.tensor.matmul(
                                    acc[:, 0:RCW], wq_sb[:, kt, ct * P:(ct + 1) * P],
                                    xqT[:, kt, rc * RCW:(rc + 1) * RCW],
                                    start=(kt == 0), stop=(kt == CT - 1))
                            nc.scalar.activation(
                                qT[:, ct, rc * RCW:(rc + 1) * RCW], acc[:, 0:RCW],
                                Act.Identity, bias=vecs_sb[:, ct, 0:1])

                    # ---------- phase 2b: identity branch (U matmul) + LN ---
                    for rc in range(RC):
                        rs = slice(rc * RCW, (rc + 1) * RCW)
                        ut_t = []
                        for mt in range(MT):
                            u1 = ld2.tile([P, RCW], f16, tag="uld", bufs=10)
                            nc.sync.dma_start(out=u1[:MTW, :],
                                              in_=utd[mt * MTW:(mt + 1) * MTW, rs])
                            ut_t.append(u1)
                        idr = apool.tile([P, CT, RCW], f16, tag="idr", bufs=2)
                        idsq = apool.tile([P, CT, RCW], f16, tag="idsq", bufs=2)
                        for cp in range(2):
                            idp = psA.tile([P, 2, BW], f32, tag="sc", bufs=3)
                            for k in range(2):
                                ct = 2 * cp + k
                                for mt in range(MT):
                                    nc.tensor.matmul(
                                        idp[:, k, 0:RCW],
                                        v_nat[:MTW, mt, ct * P:(ct + 1) * P],
                                        ut_t[mt][:MTW, :],
                                        start=(mt == 0), stop=(mt == MT - 1))
                            nc.vector.tensor_copy(
                                out=idr[:, 2 * cp:2 * cp + 2, :],
                                in_=idp[:, :, 0:RCW])
                            nc.scalar.activation(idsq[:, 2 * cp:2 * cp + 2, :],
                                                 idp[:, :, 0:RCW], Act.Square)
                        st1 = psA.tile([P, BW], f32, tag="ov", bufs=2)
                        st2 = psA.tile([P, BW], f32, tag="ov", bufs=2)
                        bc = psA.tile([P, BW], f32, tag="ov", bufs=2)
                        bc2 = psA.tile([P, BW], f32, tag="ov", bufs=2)
                        _ln_stats(nc, work, ones1_sb, ones128_sb,
                                  [idr[:, ct, :] for ct in range(CT)],
                                  [idsq[:, ct, :] for ct in range(CT)], RCW,
                                  eps_sb, st1[0:1, 0:RCW], st2[0:1, 0:RCW],
                                  bc[:, 0:RCW], bc2[:, 0:RCW])
                        for ct in range(CT):
                            t1 = work.tile([P, RCW], f32, tag="lnt")
                            nc.vector.tensor_sub(out=t1[:], in0=idr[:, ct, :],
                                                 in1=bc[:, 0:RCW])
                            t2 = work.tile([P, RCW], f32, tag="lnt2")
                            nc.vector.tensor_mul(out=t2[:], in0=t1[:],
                                                 in1=bc2[:, 0:RCW])
                            nc.vector.tensor_scalar(
                                out=lnidT[:, ct, rs], in0=t2[:],
                                scalar1=vecs_sb[:, ct, 4:5], scalar2=vecs_sb[:, ct, 5:6],
                                op0=Alu.mult, op1=Alu.add)

                # ---------- phase 3: attention (scalar engine = Exp only) ---
                for rc in range(RC):
                    rs = slice(rc * RCW, (rc + 1) * RCW)
                    oT65 = apool.tile([P, H, RCW], f16, tag="ot65", bufs=2)
                    for hh in range(H):
                        pb = HD * (hh % 2)
                        ci = hh // 2
                        pT = ppool.tile([P, MT, BW], f16, tag="pt")
                        ov = psA.tile([P, BW], f32, tag="ov", bufs=2)

                        def ov_mm(mt):
                            nc.tensor.matmul(
                                ov[0:HD + 1, 0:RCW], v_aug[:MTW, mt, hh, :],
                                pT[:MTW, mt, 0:RCW],
                                start=(mt == 0), stop=(mt == MT - 1))

                        for g in range(4):
                            sc = psA.tile([P, 2, BW], f32, tag="sc", bufs=3)
                            for k in range(2):
                                mt = 2 * g + k
                                nc.tensor.matmul(
                                    sc[:MTW, k, 0:RCW],
                                    kT[pb:pb + HD, ci, mt * MTW:(mt + 1) * MTW],
                                    qT[pb:pb + HD, ci, rs],
                                    start=True, stop=True)
                            nc.scalar.activation(pT[:MTW, 2 * g:2 * g + 2, :],
                                                 sc[:MTW, :, :], Act.Exp,
                                                 scale=SCALE)
                            # trail ov accumulation one exp group behind the
                            # score stream so the PE never waits on the ACT
                            if g >= 1:
                                ov_mm(2 * (g - 1))
                                ov_mm(2 * (g - 1) + 1)
                        for mt in (6, 7):
                            ov_mm(mt)
                        nc.vector.tensor_copy(out=oT65[0:HD + 1, hh, :],
                                              in_=ov[0:HD + 1, 0:RCW])

                    # --- normalize + add identity + output projection ---
                    den8 = apool.tile([H, RCW], f16, tag="den8", bufs=2)
                    for hh in range(H):
                        nc.sync.dma_start(out=den8[hh:hh + 1, :],
                                          in_=oT65[HD:HD + 1, hh, :])
                    rec8 = apool.tile([H, RCW], f16, tag="rec8", bufs=2)
                    nc.vector.reciprocal(out=rec8[:], in_=den8[:])
                    sum_r = apool.tile([P, CT, RCW], f16, tag="sumr", bufs=2)
                    for ct in range(CT):
                        recB = psA.tile([P, BW], f32, tag="ov", bufs=2)
                        nc.tensor.matmul(recB[:, 0:RCW],
                                         e8_sb[:, ct * P:(ct + 1) * P],
                                         rec8[:], start=True, stop=True)
                        tmp = work.tile([P, RCW], f32, tag="ntmp")
                        nc.vector.tensor_mul(out=tmp[0:HD, :],
                                             in0=oT65[0:HD, 2 * ct, :],
                                             in1=recB[0:HD, 0:RCW])
                        nc.vector.tensor_mul(out=tmp[HD:P, :],
                                             in0=oT65[0:HD, 2 * ct + 1, :],
                                             in1=recB[HD:P, 0:RCW])
                        nc.vector.tensor_add(out=sum_r[:, ct, :], in0=tmp[:],
                                             in1=lnidT[:, ct, rs])
                    for ct2 in range(CT):
                        fin = psA.tile([P, BW], f32, tag="ov", bufs=2)
                        for kt in range(CT):
                            nc.tensor.matmul(
                                fin[:, 0:RCW],
                                wp_sb[:, kt, ct2 * P:(ct2 + 1) * P],
                                sum_r[:, kt, :],
                                start=(kt == 0), stop=(kt == CT - 1))
                        oF = apool.tile([P, RCW], f16, tag="of", bufs=2)
                        nc.vector.tensor_scalar_add(out=oF[:], in0=fin[:, 0:RCW],
                                                    scalar1=vecs_sb[:, ct2, 6:7])
                        nc.sync.dma_start(out=yt[ct2 * P:(ct2 + 1) * P, rs], in_=oF[:])

    nc.finalize()
    return nc


def _parity_perm():
    perm = np.empty(N, np.int64)
    for a in range(2):
        for b in range(2):
            for c in range(2):
                blk = (a * 4 + b * 2 + c) * NSR
                for d in range(DR):
                    for h in range(DR):
                        for w_ in range(DR):
                            perm[blk + d * 100 + h * 10 + w_] = (
                                (2 * d + a) * 400 + (2 * h + b) * 20 + (2 * w_ + c))
    return perm


def _host_consts():
    eye = np.eye(P, dtype=np.float16)
    e8 = np.zeros((H, C), np.float16)
    for p in range(C):
        hh = 2 * (p // P) + (p % P) // HD
        e8[hh, p] = 1.0
    ones1 = np.ones((P, 1), np.float16)
    ones128 = np.ones((1, P), np.float16)
    epsv = np.full((P, 1), EPS, np.float32)
    return eye, e8, ones1, ones128, epsv


def _interp_1d(n_out, n_in, off):
    out = []
    for i in range(n_out):
        src = (off + i + 0.5) / 2.0 - 0.5
        lo = int(np.floor(src))
        f = src - lo
        lo_c = min(max(lo, 0), n_in - 1)
        hi_c = min(max(lo + 1, 0), n_in - 1)
        out.append(((lo_c, 1.0 - f), (hi_c, f)))
    return out


def _build_ut(j):
    """U^T (NSR, NCHUNK): idT[:, n] = sum_m v_nat[m, :] * UT[m, n], quarter j."""
    ut = np.zeros((NSR, NCHUNK), np.float32)
    d_lo = (j * NCHUNK) // (D3 * D3)
    dmap = _interp_1d(5, DR, d_lo)
    hmap = _interp_1d(D3, DR, 0)
    wmap = _interp_1d(D3, DR, 0)
    for dd in range(5):
        for hh2 in range(D3):
            for ww in range(D3):
                nloc = dd * D3 * D3 + hh2 * D3 + ww
                for (di, dwt) in dmap[dd]:
                    for (hi, hwt) in hmap[hh2]:
                        for (wi, wwt) in wmap[ww]:
                            m = di * DR * DR + hi * DR + wi
                            ut[m, nloc] += dwt * hwt * wwt
    return ut.astype(np.float16)


def _build_wdiag(sr_w):
    """Per-tap diagonal weight matrices: [CT, NTAP, P, P] fp16.

    Slots 0..26 are the 27 conv taps; slots 27..29 are the negated weights
    of the three dh==-1,dw==-1 taps (t = 0, 9, 18) for wrap compensation.
    """
    w27 = sr_w.reshape(C, 27).astype(np.float32)
    d = np.zeros((CT, NTAP, P, P), np.float16)
    idx = np.arange(P)
    for ct in range(CT):
        blk = w27[ct * P:(ct + 1) * P]        # (P, 27)
        d[ct, :27, idx, idx] = blk.astype(np.float16)
        for t, slot in _NEG.items():
            d[ct, slot, idx, idx] = (-blk[:, t]).astype(np.float16)
    return np.ascontiguousarray(d)


def kernel(**inputs):
    global _PROGRAM, _HOST, LAST_RESULT
    x = np.asarray(inputs["x"], np.float32)
    Wq = np.asarray(inputs["Wq"], np.float32)
    bq = np.asarray(inputs["bq"], np.float32)
    Wkv = np.asarray(inputs["Wkv"], np.float32)
    bkv_ = np.asarray(inputs["bkv"], np.float32)
    sr_w = np.asarray(inputs["sr_w"], np.float32)
    sr_b = np.asarray(inputs["sr_b"], np.float32)
    sr_g = np.asarray(inputs["sr_g"], np.float32)
    sr_beta = np.asarray(inputs["sr_beta"], np.float32)
    up_g = np.asarray(inputs["up_g"], np.float32)
    up_beta = np.asarray(inputs["up_beta"], np.float32)
    Wp = np.asarray(inputs["Wp"], np.float32)
    bp = np.asarray(inputs["bp"], np.float32)

    if _PROGRAM is None:
        _PROGRAM = _build_program()
    nc = _PROGRAM

    if _HOST is None:
        _HOST = (_host_consts(), [_build_ut(j) for j in range(4)], _parity_perm())
    (eye, e8, ones1, ones128, epsv), uts, perm = _HOST

    wdiag = _build_wdiag(sr_w)
    vecs = np.ascontiguousarray(
        np.stack([bq, sr_b, sr_g, sr_beta, up_g, up_beta, bp], axis=1))
    wq16 = np.ascontiguousarray(Wq.astype(np.float16))
    wkv16 = np.ascontiguousarray(Wkv.astype(np.float16))
    wp16 = np.ascontiguousarray(Wp.astype(np.float16))

    xtds, xqts = [], []
    for b in range(B):
        xtds.append(np.ascontiguousarray(
            x[b][perm].T.reshape(CT, P, N).astype(np.float16)))
        xqts.append([np.ascontiguousarray(
            x[b, j * NCHUNK:(j + 1) * NCHUNK].T.reshape(CT, P, NCHUNK)
            .astype(np.float16)) for j in range(4)])

    in_maps = []
    for core in range(8):
        b, j = core // 4, core % 4
        in_maps.append({
            "xtd": xtds[b],
            "xqt": xqts[b][j],
            "wq": wq16, "wkv": wkv16, "wp": wp16,
            "wdiag": wdiag, "vecs": vecs, "bkv": bkv_,
            "ut": uts[j],
            "eye": eye, "e8": e8, "ones1": ones1, "ones128": ones128,
            "epsv": epsv,
        })

    res = run_bass_kernel_spmd(nc, in_maps, core_ids=list(range(8)), trace=TRACE)
    LAST_RESULT = res
    out = np.empty((B, N, C), np.float32)
    for core in range(8):
        b, j = core // 4, core % 4
        out[b, j * NCHUNK:(j + 1) * NCHUNK, :] = (
            res.results[core]["yt"].astype(np.float32).T)
    return out


# revision 17
# speedup vs baseline: 1.1708x; 1.0826x over previous
"""Sparse-attention (PVT-style SRA) kernel for 8 Trainium2 NeuronCores.

Sharding: 8 cores = 2 batches x 4 row-quarters of N=8000. Each core computes
its 2000 output rows end-to-end; the spatial-reduction branch (conv+LN+kv) is
replicated per batch. All matmuls run in fp16 (1 cycle/row on the PE, fast
weight load); accumulation stays fp32 in PSUM. The depthwise conv runs on the
tensor engine as diagonal-weight matmuls accumulating all 27 taps in PSUM;
the trilinear upsample is an interpolation matmul whose weights (products of
1/4 and 3/4) are fp16-exact. Host pre-transposes x into channel-major layout
so the kernel does no PE transposes of the input.

Phases: (1) conv + LN + kv + v transposes, (2) q projection + upsampled
identity branch + LN, (3) attention with an Exp-only scalar engine, then
normalize + output projection per row chunk.
"""

import sys

sys.path.insert(0, "/opt/trn_rl_repo")

import contextlib
import numpy as np
import concourse.bacc as bacc
import concourse.mybir as mybir
from concourse.tile import TileContext
from concourse.bass_utils import run_bass_kernel_spmd

dt = mybir.dt
Alu = mybir.AluOpType
Act = mybir.ActivationFunctionType

P = 128
B, N, C = 2, 8000, 512
H, HD = 8, 64
D3 = 20          # full spatial edge (D=H=W)
DR = 10          # reduced spatial edge
NSR = 1000       # DR**3
CT = 4           # C // P
NCHUNK = 2000    # output rows per core
RC = 4           # row chunks per core
RCW = 500        # rows per chunk
MT = 8           # key tiles
MTW = 125        # keys per tile
BW = 512         # PSUM bank width (f32 elems)
NTAP = 30        # 27 taps + 3 negated wrap-compensation taps
SCALE = HD ** -0.5
EPS = 1e-6

# dh==-1 and dw==-1 taps get wrap compensation with negated weights at
# diag slots 27..29 (tap index t -> neg slot).
_NEG = {0: 27, 9: 28, 18: 29}

_PROGRAM = None
_HOST = None
TRACE = False
LAST_RESULT = None


def _conv_ops(xf, acc_h, h):
    """Matmul op list for one 500-col half of the conv output block.

    The host permutes x's spatial rows into parity-block order: flat index
    (a*4+b*2+c)*1000 + d*100 + h*10 + w  <->  original (2d+a, 2h+b, 2w+c).
    Every tap then reduces to a shifted access pattern on xf, and the conv
    becomes a sum over taps of diag(w_tap) @ x_shifted, accumulated in PSUM.
    The center tap comes first and covers the whole half, so it owns
    start=True. Returns [(out_ap, in_ap, diag_slot)].

    xf: (p, 8000) fp16 input view; acc_h: (p, 500) f32 PSUM AP for output
    columns [h*500, (h+1)*500) of the 1000-col reduced block.
    """
    c0 = h * RCW
    c1 = c0 + RCW
    ops = []
    taps = [(0, 0, 0)] + [(a, b, c)
                          for a in (-1, 0, 1) for b in (-1, 0, 1) for c in (-1, 0, 1)
                          if (a, b, c) != (0, 0, 0)]
    for (dd, dh, dw) in taps:
        pa, pb, pc = (0 if dd == 0 else 1), (0 if dh == 0 else 1), (0 if dw == 0 else 1)
        Dd, Dh = (-1 if dd == -1 else 0), (-1 if dh == -1 else 0)
        bb = (pa * 4 + pb * 2 + pc) * 1000
        d0 = 1 if dd == -1 else 0
        t = (dd + 1) * 9 + (dh + 1) * 3 + (dw + 1)
        if dh != -1 and dw != -1:
            # contiguous: out o gets in bb + Dd*100 + o, o in [d0*100, 1000)
            a0 = max(c0, d0 * 100)
            ib = bb + Dd * 100 + a0
            ops.append((acc_h[:, a0 - c0:c1 - c0], xf[:, ib:ib + (c1 - a0)], t))
        elif dw == -1 and dh != -1:
            # out (x, w in 1..10), x = d*10+h_ in [d0*10, 100);
            # in = bb + Dd*100 + x*10 + (w-1)
            x0 = max(c0 // 10, d0 * 10)
            x1 = c1 // 10
            o = acc_h.rearrange("p (x w) -> p x w", w=10)[
                :, x0 - c0 // 10:x1 - c0 // 10, 1:10]
            ib = bb + Dd * 100 + x0 * 10
            i = xf[:, ib:ib + (x1 - x0) * 10].rearrange(
                "p (x w) -> p x w", w=10)[:, :, 0:9]
            ops.append((o, i, t))
        elif dh == -1 and dw != -1:
            # out (d, r in 10..100), d in [d0, 10); in = bb + Dd*100 + d*100 + r-10
            dlo = max(c0 // 100, d0)
            dhi = c1 // 100
            o = acc_h.rearrange("p (d r) -> p d r", r=100)[
                :, dlo - c0 // 100:dhi - c0 // 100, 10:100]
            ib = bb + Dd * 100 + dlo * 100
            i = xf[:, ib:ib + (dhi - dlo) * 100].rearrange(
                "p (d r) -> p d r", r=100)[:, :, 0:90]
            ops.append((o, i, t))
        else:
            # dh==-1 and dw==-1: extended full-block op + 2 negated comps
            s = bb + Dd * 100 - 11
            a0 = max(c0, d0 * 100)
            ops.append((acc_h[:, a0 - c0:c1 - c0], xf[:, a0 + s:c1 + s], t))
            dlo = max(c0 // 100, d0)
            dhi = c1 // 100
            ov = acc_h.rearrange("p (d q w) -> p d q w", q=10, w=10)
            iv = xf[:, dlo * 100 + s:dhi * 100 + s].rearrange(
                "p (d q w) -> p d q w", q=10, w=10)
            dsl = slice(dlo - c0 // 100, dhi - c0 // 100)
            # comp1: out (d, q full, w=0) wrongly read (d, q-1, 9)
            ops.append((ov[:, dsl, :, 0], iv[:, :, :, 0], _NEG[t]))
            # comp2: out (d, q=0, w 1..9) wrongly read (d-1, 9, w-1)
            ops.append((ov[:, dsl, 0, 1:10], iv[:, :, 0, 1:10], _NEG[t]))
    return ops


def _ln_stats(nc, work, ones1_sb, ones128_sb, x_tiles, sq_tiles, width,
              eps_sb, sx, sxx, muB, rstdB):
    """Cross-partition LayerNorm stats for C=512 split over 4 partition tiles.

    x_tiles/sq_tiles: lists of 4 fp16 APs, each (128, width). sx/sxx are
    (1, width) f32 PSUM APs; muB/rstdB are (128, width) f32 PSUM APs that
    receive the broadcast mean / inverse-std.
    """
    f32, f16 = dt.float32, dt.float16
    n = len(x_tiles)
    for i, xt in enumerate(x_tiles):
        nc.tensor.matmul(sx, ones1_sb[:], xt, start=(i == 0), stop=(i == n - 1))
    for i, st in enumerate(sq_tiles):
        nc.tensor.matmul(sxx, ones1_sb[:], st, start=(i == 0), stop=(i == n - 1))
    mu_r = work.tile([1, width], f16, tag="mu", bufs=2)
    nc.vector.tensor_scalar_mul(out=mu_r[:], in0=sx, scalar1=1.0 / C)
    msq = work.tile([1, width], f32, tag="msq", bufs=2)
    nc.vector.tensor_scalar_mul(out=msq[:], in0=sxx, scalar1=1.0 / C)
    mu2 = work.tile([1, width], f32, tag="mu2", bufs=2)
    nc.vector.tensor_mul(out=mu2[:], in0=mu_r[:], in1=mu_r[:])
    var = work.tile([1, width], f32, tag="var", bufs=2)
    nc.vector.tensor_sub(out=var[:], in0=msq[:], in1=mu2[:])
    std = work.tile([1, width], f32, tag="std", bufs=2)
    nc.scalar.activation(std[:], var[:], Act.Sqrt, bias=eps_sb[0:1, 0:1])
    rstd_r = work.tile([1, width], f16, tag="rstd", bufs=2)
    nc.vector.reciprocal(out=rstd_r[:], in_=std[:])
    nc.tensor.matmul(muB, ones128_sb[:], mu_r[:], start=True, stop=True)
    nc.tensor.matmul(rstdB, ones128_sb[:], rstd_r[:], start=True, stop=True)


def _build_program():
    nc = bacc.Bacc("TRN2", target_bir_lowering=False, debug=False, num_devices=8)
    f32, f16 = dt.float32, dt.float16

    xqtd = nc.dram_tensor("xqt", [CT, P, NCHUNK], f16, kind="ExternalInput").ap()
    xtd = nc.dram_tensor("xtd", [CT, P, N], f16, kind="ExternalInput").ap()
    wq = nc.dram_tensor("wq", [C, C], f16, kind="ExternalInput").ap()
    wkv = nc.dram_tensor("wkv", [C, 2 * C], f16, kind="ExternalInput").ap()
    wp = nc.dram_tensor("wp", [C, C], f16, kind="ExternalInput").ap()
    wdiagd = nc.dram_tensor("wdiag", [CT, NTAP, P, P], f16, kind="ExternalInput").ap()
    vecsd = nc.dram_tensor("vecs", [C, 7], f32, kind="ExternalInput").ap()
    bkvd = nc.dram_tensor("bkv", [2 * C], f32, kind="ExternalInput").ap()
    utd = nc.dram_tensor("ut", [NSR, NCHUNK], f16, kind="ExternalInput").ap()
    eyed = nc.dram_tensor("eye", [P, P], f16, kind="ExternalInput").ap()
    e8d = nc.dram_tensor("e8", [H, C], f16, kind="ExternalInput").ap()
    ones1d = nc.dram_tensor("ones1", [P, 1], f16, kind="ExternalInput").ap()
    ones128d = nc.dram_tensor("ones128", [1, P], f16, kind="ExternalInput").ap()
    epsd = nc.dram_tensor("epsv", [P, 1], f32, kind="ExternalInput").ap()
    yt = nc.dram_tensor("yt", [C, NCHUNK], f16, kind="ExternalOutput").ap()

    with TileContext(nc) as tc, nc.allow_low_precision(
            reason="fp16 data with fp32 PSUM accumulation; tol is 2e-2"):
        with contextlib.ExitStack() as octx:
            consts = octx.enter_context(tc.tile_pool(name="consts", bufs=1))
            keep1 = octx.enter_context(tc.tile_pool(name="keep1", bufs=1))
            work = octx.enter_context(tc.tile_pool(name="work", bufs=2))

            # ---------- constants ----------
            eye_sb = consts.tile([P, P], f16)
            nc.sync.dma_start(out=eye_sb[:], in_=eyed[:])
            e8_sb = consts.tile([H, C], f16)
            nc.sync.dma_start(out=e8_sb[:], in_=e8d[:])
            ones1_sb = consts.tile([P, 1], f16)
            nc.sync.dma_start(out=ones1_sb[:], in_=ones1d[:])
            ones128_sb = consts.tile([1, P], f16)
            nc.sync.dma_start(out=ones128_sb[:], in_=ones128d[:])
            eps_sb = consts.tile([P, 1], f32)
            nc.sync.dma_start(out=eps_sb[:], in_=epsd[:])
            vecs_sb = consts.tile([P, CT, 7], f32)
            nc.sync.dma_start(out=vecs_sb[:], in_=vecsd.rearrange("(o p) t -> p o t", p=P))
            bkv_sb = consts.tile([P, 2 * CT], f32)
            nc.sync.dma_start(out=bkv_sb[:], in_=bkvd.rearrange("(o p) -> p o", p=P))

            qT = keep1.tile([P, CT, NCHUNK], f16)       # 16 KB/part
            kT = keep1.tile([P, CT, NSR], f16)          # 8 KB/part
            lnidT = keep1.tile([P, CT, NCHUNK], f16)    # 16 KB/part
            wp_sb = keep1.tile([P, CT, C], f16)
            v_nat = keep1.tile([P, MT, C], f16)
            v_aug = keep1.tile([P, MT, H, HD + 1], f16)

            # ================= phase 1: conv + LN + kv + v ==================
            with contextlib.ExitStack() as ectx:
                psE = ectx.enter_context(tc.tile_pool(name="psE", bufs=1, space="PSUM"))

                with tc.tile_pool(name="cpool", bufs=1) as cpool, \
                        tc.tile_pool(name="dgp", bufs=1) as dgp:
                    diag_sb = dgp.tile([P, CT, NTAP, P], f16)   # 30 KB/part
                    nc.sync.dma_start(
                        out=diag_sb[:],
                        in_=wdiagd.rearrange("c t k m -> k c t m"))
                    wkv_sb = dgp.tile([P, CT, 2 * C], f16)
                    nc.sync.dma_start(out=wkv_sb[:],
                                      in_=wkv.rearrange("(k p) m -> p k m", p=P))
                    nc.sync.dma_start(out=wp_sb[:],
                                      in_=wp.rearrange("(k p) m -> p k m", p=P))

                    # ---------- conv on the PE, evac+bias+square on scalar ---
                    xr = cpool.tile([P, CT, NSR], f16)
                    sq = cpool.tile([P, CT, NSR], f16)
                    for ct in range(CT):
                        xct = cpool.tile([P, N], f16, tag="xct", bufs=2)
                        nc.sync.dma_start(out=xct[:], in_=xtd[ct, :, :])
                        acc = psE.tile([P, 2, BW], f32, tag="e", bufs=3)
                        for hf in range(2):
                            ops = _conv_ops(xct[:], acc[:, hf, 0:RCW], hf)
                            for i, (o, inp, t) in enumerate(ops):
                                nc.tensor.matmul(
                                    o, diag_sb[:, ct, t, :], inp,
                                    start=(i == 0), stop=(i == len(ops) - 1))
                        cs = slice(0, NSR)
                        nc.scalar.activation(xr[:, ct, :], acc[:, :, 0:RCW],
                                             Act.Identity, bias=vecs_sb[:, ct, 1:2])
                        nc.scalar.activation(sq[:, ct, :], acc[:, :, 0:RCW],
                                             Act.Square, bias=vecs_sb[:, ct, 1:2])

                    # ---------- LayerNorm over C -> xrn ----------
                    xrn = cpool.tile([P, CT, NSR], f16)
                    for ch in range(2):
                        cs = slice(ch * RCW, (ch + 1) * RCW)
                        st = psE.tile([P, 2, BW], f32, tag="e", bufs=3)
                        bc = psE.tile([P, 2, BW], f32, tag="e", bufs=3)
                        _ln_stats(nc, work, ones1_sb, ones128_sb,
                                  [xr[:, ct, cs] for ct in range(CT)],
                                  [sq[:, ct, cs] for ct in range(CT)], RCW,
                                  eps_sb, st[0:1, 0, 0:RCW], st[0:1, 1, 0:RCW],
                                  bc[:, 0, 0:RCW], bc[:, 1, 0:RCW])
                        for ct in range(CT):
                            t1 = work.tile([P, RCW], f32, tag="lnt")
                            nc.vector.tensor_sub(out=t1[:], in0=xr[:, ct, cs],
                                                 in1=bc[:, 0, 0:RCW])
                            nc.vector.tensor_mul(out=xrn[:, ct, cs], in0=t1[:],
                                                 in1=bc[:, 1, 0:RCW])

                    # ---------- kv projection ----------
                    vT = cpool.tile([P, CT, NSR], f16)
                    for mt8 in range(2 * CT):
                        dsts = kT if mt8 < CT else vT
                        di = mt8 if mt8 < CT else mt8 - CT
                        acc = psE.tile([P, 2, BW], f32, tag="e", bufs=3)
                        for ch in range(2):
                            cs = slice(ch * RCW, (ch + 1) * RCW)
                            for kt in range(CT):
                                nc.tensor.matmul(
                                    acc[:, ch, 0:RCW],
                                    wkv_sb[:, kt, mt8 * P:(mt8 + 1) * P],
                                    xrn[:, kt, cs],
                                    start=(kt == 0), stop=(kt == CT - 1))
                        if mt8 % 2 == 0:
                            nc.vector.tensor_scalar_add(
                                out=dsts[:, di, :],
                                in0=acc[:, :, 0:RCW],
                                scalar1=bkv_sb[:, mt8:mt8 + 1])
                        else:
                            nc.scalar.activation(
                                dsts[:, di, :], acc[:, :, 0:RCW],
                                Act.Identity, bias=bkv_sb[:, mt8:mt8 + 1])

                    # ---------- v natural + ones column (v_aug) ----------
                    nc.gpsimd.tensor_copy(
                        out=v_aug[:, :, :, HD:HD + 1],
                        in_=ones1_sb[:, 0:1, None, None].to_broadcast([P, MT, H, 1]))
                    for ci in range(CT):
                        for mt in range(MT):
                            tp = psE.tile([P, P], f16, tag="trps", bufs=2)
                            nc.tensor.transpose(
                                tp[:MTW, :], vT[:, ci, mt * MTW:(mt + 1) * MTW],
                                eye_sb[:])
                            nc.vector.tensor_copy(
                                out=v_nat[:MTW, mt, ci * P:(ci + 1) * P],
                                in_=tp[:MTW, :])
                            nc.gpsimd.tensor_copy(
                                out=v_aug[:MTW, mt, 2 * ci, 0:HD],
                                in_=v_nat[:MTW, mt, ci * P:ci * P + HD])
                            nc.gpsimd.tensor_copy(
                                out=v_aug[:MTW, mt, 2 * ci + 1, 0:HD],
                                in_=v_nat[:MTW, mt, ci * P + HD:(ci + 1) * P])

            # ============ phases 2+3: q proj + identity interleaved into the
            # attention loop as filler bundles; one wide SBUF exp per head ====
            with contextlib.ExitStack() as actx:
                psA = actx.enter_context(tc.tile_pool(name="psA", bufs=1, space="PSUM"))
                ld2 = actx.enter_context(tc.tile_pool(name="ld2", bufs=2))
                ppool = actx.enter_context(tc.tile_pool(name="ppool", bufs=2))
                spool = actx.enter_context(tc.tile_pool(name="spool", bufs=2))
                apool = actx.enter_context(tc.tile_pool(name="apool", bufs=1))
                wqp = actx.enter_context(tc.tile_pool(name="wqp", bufs=1))

                wq_sb = wqp.tile([P, CT, C], f16)
                nc.sync.dma_start(out=wq_sb[:],
                                  in_=wq.rearrange("(k p) m -> p k m", p=P))
                xqT = wqp.tile([P, CT, NCHUNK], f16)
                for ct in range(CT):
                    nc.sync.dma_start(out=xqT[:, ct, :], in_=xqtd[ct, :, :])

                def mk_bundles(rc):
                    """Phase-2 work for row chunk rc as small closures that the
                    attention loop of chunk rc-1 drains into its PE/DVE slack."""
                    rs = slice(rc * RCW, (rc + 1) * RCW)
                    st = {}
                    bundles = []

                    def b_q(ct):
                        acc = psA.tile([P, BW], f32, tag="ov", bufs=2)
                        for kt in range(CT):
                            nc.tensor.matmul(
                                acc[:, 0:RCW], wq_sb[:, kt, ct * P:(ct + 1) * P],
                                xqT[:, kt, rs],
                                start=(kt == 0), stop=(kt == CT - 1))
                        nc.scalar.activation(
                            qT[:, ct, rs], acc[:, 0:RCW],
                            Act.Identity, bias=vecs_sb[:, ct, 0:1])
                    for ct in range(CT):
                        bundles.append(lambda ct=ct: b_q(ct))

                    def b_ut():
                        st["ut"] = []
                        for mt in range(MT):
                            u1 = ld2.tile([P, RCW], f16, tag="uld", bufs=10)
                            nc.sync.dma_start(out=u1[:MTW, :],
                                              in_=utd[mt * MTW:(mt + 1) * MTW, rs])
                            st["ut"].append(u1)
                    bundles.append(b_ut)

                    def b_idp(cp, k):
                        if k == 0:
                            st["idp", cp] = psA.tile([P, 2, BW], f32, tag="sc",
                                                     bufs=3, name=f"idp{cp}")
                        idp = st["idp", cp]
                        ct = 2 * cp + k
                        for mt in range(MT):
                            nc.tensor.matmul(
                                idp[:, k, 0:RCW],
                                v_nat[:MTW, mt, ct * P:(ct + 1) * P],
                                st["ut"][mt][:MTW, :],
                                start=(mt == 0), stop=(mt == MT - 1))

                    def b_idev(cp):
                        if cp == 0:
                            st["idr"] = apool.tile([P, CT, RCW], f16, tag="idr",
                                                   bufs=2, name="idr")
                            st["idsq"] = apool.tile([P, CT, RCW], f16, tag="idsq",
                                                    bufs=2, name="idsq")
                        idp = st["idp", cp]
                        nc.vector.tensor_copy(out=st["idr"][:, 2 * cp:2 * cp + 2, :],
                                              in_=idp[:, :, 0:RCW])
                        nc.scalar.activation(st["idsq"][:, 2 * cp:2 * cp + 2, :],
                                             idp[:, :, 0:RCW], Act.Square)
                    for cp in range(2):
                        bundles.append(lambda cp=cp: b_idp(cp, 0))
                        bundles.append(lambda cp=cp: b_idp(cp, 1))
                        bundles.append(lambda cp=cp: b_idev(cp))

                    def b_stats():
                        idr, idsq = st["idr"], st["idsq"]
                        st1 = psA.tile([P, BW], f32, tag="ov", bufs=2)
                        st2 = psA.tile([P, BW], f32, tag="ov", bufs=2)
                        bc = psA.tile([P, BW], f32, tag="ov", bufs=2)
                        bc2 = psA.tile([P, BW], f32, tag="ov", bufs=2)
                        st["bc"], st["bc2"] = bc, bc2
                        _ln_stats(nc, work, ones1_sb, ones128_sb,
                                  [idr[:, ct, :] for ct in range(CT)],
                                  [idsq[:, ct, :] for ct in range(CT)], RCW,
                                  eps_sb, st1[0:1, 0:RCW], st2[0:1, 0:RCW],
                                  bc[:, 0:RCW], bc2[:, 0:RCW])
                    bundles.append(b_stats)

                    def b_apply(cp):
                        bc, bc2 = st["bc"], st["bc2"]
                        for ct in (2 * cp, 2 * cp + 1):
                            t1 = work.tile([P, RCW], f32, tag="lnt")
                            nc.vector.tensor_sub(out=t1[:], in0=st["idr"][:, ct, :],
                                                 in1=bc[:, 0:RCW])
                            t2 = work.tile([P, RCW], f32, tag="lnt2")
                            nc.vector.tensor_mul(out=t2[:], in0=t1[:],
                                                 in1=bc2[:, 0:RCW])
                            nc.vector.tensor_scalar(
                                out=lnidT[:, ct, rs], in0=t2[:],
                                scalar1=vecs_sb[:, ct, 4:5],
                                scalar2=vecs_sb[:, ct, 5:6],
                                op0=Alu.mult, op1=Alu.add)
                    for cp in range(2):
                        bundles.append(lambda cp=cp: b_apply(cp))
                    return bundles

                # phase 2 for the first row chunk runs up front
                for b in mk_bundles(0):
                    b()

                for rc in range(RC):
                    rs = slice(rc * RCW, (rc + 1) * RCW)
                    oT65 = apool.tile([P, H, RCW], f16, tag="ot65", bufs=2)
                    bundles = mk_bundles(rc + 1) if rc + 1 < RC else []
                    bi = 0
                    prev = None

                    def emit_ov(p):
                        pTp, hhp = p
                        ov = psA.tile([P, BW], f32, tag="ov", bufs=2)
                        for mt in range(MT):
                            nc.tensor.matmul(
                                ov[0:HD + 1, 0:RCW], v_aug[:MTW, mt, hhp, :],
                                pTp[:MTW, mt, 0:RCW],
                                start=(mt == 0), stop=(mt == MT - 1))
                        nc.vector.tensor_copy(out=oT65[0:HD + 1, hhp, :],
                                              in_=ov[0:HD + 1, 0:RCW])

                    for hh in range(H):
                        pb = HD * (hh % 2)
                        ci = hh // 2
                        pT = ppool.tile([P, MT, BW], f16, tag="pt")
                        se = spool.tile([P, MT, BW], f16, tag="se")
                        for g in range(4):
                            sc = psA.tile([P, 2, BW], f32, tag="sc", bufs=3)
                            for k in range(2):
                                mt = 2 * g + k
                                nc.tensor.matmul(
                                    sc[:MTW, k, 0:RCW],
                                    kT[pb:pb + HD, ci, mt * MTW:(mt + 1) * MTW],
                                    qT[pb:pb + HD, ci, rs],
                                    start=True, stop=True)
                            nc.vector.tensor_copy(
                                out=se[:MTW, 2 * g:2 * g + 2, :],
                                in_=sc[:MTW, :, :])
                        nc.scalar.activation(pT[:MTW, :, :], se[:MTW, :, :],
                                             Act.Exp, scale=SCALE)
                        if prev is not None:
                            emit_ov(prev)
                        prev = (pT, hh)
                        quota = (hh + 1) * len(bundles) // H
                        while bi < quota:
                            bundles[bi]()
                            bi += 1
                    emit_ov(prev)
                    while bi < len(bundles):
                        bundles[bi]()
                        bi += 1

                    # --- normalize + add identity + output projection ---
                    den8 = apool.tile([H, RCW], f16, tag="den8", bufs=2)
                    for hh in range(H):
                        nc.sync.dma_start(out=den8[hh:hh + 1, :],
                                          in_=oT65[HD:HD + 1, hh, :])
                    rec8 = apool.tile([H, RCW], f16, tag="rec8", bufs=2)
                    nc.vector.reciprocal(out=rec8[:], in_=den8[:])
                    sum_r = apool.tile([P, CT, RCW], f16, tag="sumr", bufs=2)
                    for ct in range(CT):
                        recB = psA.tile([P, BW], f32, tag="ov", bufs=2)
                        nc.tensor.matmul(recB[:, 0:RCW],
                                         e8_sb[:, ct * P:(ct + 1) * P],
                                         rec8[:], start=True, stop=True)
                        tmp = work.tile([P, RCW], f32, tag="ntmp")
                        nc.vector.tensor_mul(out=tmp[0:HD, :],
                                             in0=oT65[0:HD, 2 * ct, :],
                                             in1=recB[0:HD, 0:RCW])
                        nc.vector.tensor_mul(out=tmp[HD:P, :],
                                             in0=oT65[0:HD, 2 * ct + 1, :],
                                             in1=recB[HD:P, 0:RCW])
                        nc.vector.tensor_add(out=sum_r[:, ct, :], in0=tmp[:],
                                             in1=lnidT[:, ct, rs])
                    for ct2 in range(CT):
                        fin = psA.tile([P, BW], f32, tag="ov", bufs=2)
                        for kt in range(CT):
                            nc.tensor.matmul(
                                fin[:, 0:RCW],
                                wp_sb[:, kt, ct2 * P:(ct2 + 1) * P],
                                sum_r[:, kt, :],
                                start=(kt == 0), stop=(kt == CT - 1))
                        oF = apool.tile([P, RCW], f16, tag="of", bufs=2)
                        nc.vector.tensor_scalar_add(out=oF[:], in0=fin[:, 0:RCW],
                                                    scalar1=vecs_sb[:, ct2, 6:7])
                        nc.sync.dma_start(out=yt[ct2 * P:(ct2 + 1) * P, rs], in_=oF[:])

    nc.finalize()
    return nc


def _parity_perm():
    perm = np.empty(N, np.int64)
    for a in range(2):
        for b in range(2):
            for c in range(2):
                blk = (a * 4 + b * 2 + c) * NSR
                for d in range(DR):
                    for h in range(DR):
                        for w_ in range(DR):
                            perm[blk + d * 100 + h * 10 + w_] = (
                                (2 * d + a) * 400 + (2 * h + b) * 20 + (2 * w_ + c))
    return perm


def _host_consts():
    eye = np.eye(P, dtype=np.float16)
    e8 = np.zeros((H, C), np.float16)
    for p in range(C):
        hh = 2 * (p // P) + (p % P) // HD
        e8[hh, p] = 1.0
    ones1 = np.ones((P, 1), np.float16)
    ones128 = np.ones((1, P), np.float16)
    epsv = np.full((P, 1), EPS, np.float32)
    return eye, e8, ones1, ones128, epsv


def _interp_1d(n_out, n_in, off):
    out = []
    for i in range(n_out):
        src = (off + i + 0.5) / 2.0 - 0.5
        lo = int(np.floor(src))
        f = src - lo
        lo_c = min(max(lo, 0), n_in - 1)
        hi_c = min(max(lo + 1, 0), n_in - 1)
        out.append(((lo_c, 1.0 - f), (hi_c, f)))
    return out


def _build_ut(j):
    """U^T (NSR, NCHUNK): idT[:, n] = sum_m v_nat[m, :] * UT[m, n], quarter j."""
    ut = np.zeros((NSR, NCHUNK), np.float32)
    d_lo = (j * NCHUNK) // (D3 * D3)
    dmap = _interp_1d(5, DR, d_lo)
    hmap = _interp_1d(D3, DR, 0)
    wmap = _interp_1d(D3, DR, 0)
    for dd in range(5):
        for hh2 in range(D3):
            for ww in range(D3):
                nloc = dd * D3 * D3 + hh2 * D3 + ww
                for (di, dwt) in dmap[dd]:
                    for (hi, hwt) in hmap[hh2]:
                        for (wi, wwt) in wmap[ww]:
                            m = di * DR * DR + hi * DR + wi
                            ut[m, nloc] += dwt * hwt * wwt
    return ut.astype(np.float16)


def _build_wdiag(sr_w):
    """Per-tap diagonal weight matrices: [CT, NTAP, P, P] fp16.

    Slots 0..26 are the 27 conv taps; slots 27..29 are the negated weights
    of the three dh==-1,dw==-1 taps (t = 0, 9, 18) for wrap compensation.
    """
    w27 = sr_w.reshape(C, 27).astype(np.float32)
    d = np.zeros((CT, NTAP, P, P), np.float16)
    idx = np.arange(P)
    for ct in range(CT):
        blk = w27[ct * P:(ct + 1) * P]        # (P, 27)
        d[ct, :27, idx, idx] = blk.astype(np.float16)
        for t, slot in _NEG.items():
            d[ct, slot, idx, idx] = (-blk[:, t]).astype(np.float16)
    return np.ascontiguousarray(d)


def kernel(**inputs):
    global _PROGRAM, _HOST, LAST_RESULT
    x = np.asarray(inputs["x"], np.float32)
    Wq = np.asarray(inputs["Wq"], np.float32)
    bq = np.asarray(inputs["bq"], np.float32)
    Wkv = np.asarray(inputs["Wkv"], np.float32)
    bkv_ = np.asarray(inputs["bkv"], np.float32)
    sr_w = np.asarray(inputs["sr_w"], np.float32)
    sr_b = np.asarray(inputs["sr_b"], np.float32)
    sr_g = np.asarray(inputs["sr_g"], np.float32)
    sr_beta = np.asarray(inputs["sr_beta"], np.float32)
    up_g = np.asarray(inputs["up_g"], np.float32)
    up_beta = np.asarray(inputs["up_beta"], np.float32)
    Wp = np.asarray(inputs["Wp"], np.float32)
    bp = np.asarray(inputs["bp"], np.float32)

    if _PROGRAM is None:
        _PROGRAM = _build_program()
    nc = _PROGRAM

    if _HOST is None:
        _HOST = (_host_consts(), [_build_ut(j) for j in range(4)], _parity_perm())
    (eye, e8, ones1, ones128, epsv), uts, perm = _HOST

    wdiag = _build_wdiag(sr_w)
    vecs = np.ascontiguousarray(
        np.stack([bq, sr_b, sr_g, sr_beta, up_g, up_beta, bp], axis=1))
    wq16 = np.ascontiguousarray(Wq.astype(np.float16))
    wkv16 = np.ascontiguousarray((sr_g[:, None] * Wkv).astype(np.float16))
    bkv_ = bkv_ + sr_beta @ Wkv
    wp16 = np.ascontiguousarray(Wp.astype(np.float16))

    xtds, xqts = [], []
    for b in range(B):
        xtds.append(np.ascontiguousarray(
            x[b][perm].T.reshape(CT, P, N).astype(np.float16)))
        xqts.append([np.ascontiguousarray(
            x[b, j * NCHUNK:(j + 1) * NCHUNK].T.reshape(CT, P, NCHUNK)
            .astype(np.float16)) for j in range(4)])

    in_maps = []
    for core in range(8):
        b, j = core // 4, core % 4
        in_maps.append({
            "xtd": xtds[b],
            "xqt": xqts[b][j],
            "wq": wq16, "wkv": wkv16, "wp": wp16,
            "wdiag": wdiag, "vecs": vecs, "bkv": bkv_,
            "ut": uts[j],
            "eye": eye, "e8": e8, "ones1": ones1, "ones128": ones128,
            "epsv": epsv,
        })

    res = run_bass_kernel_spmd(nc, in_maps, core_ids=list(range(8)), trace=TRACE)
    LAST_RESULT = res
    out = np.empty((B, N, C), np.float32)
    for core in range(8):
        b, j = core // 4, core % 4
        out[b, j * NCHUNK:(j + 1) * NCHUNK, :] = (
            res.results[core]["yt"].astype(np.float32).T)
    return out
